# revision 1
# baseline (speedup 1.0000x reference)
"""Trainium2 Bass kernel for nn_CostEstimationNet (Bayesian LSTM + linear head).

Strategy (data-parallel over batch, 8 cores, 512 batch/core):
  - Host: reparameterize weights (mu + softplus(rho)*eps), fold sigmoid into
    tanh via 0.5 pre-scaling (sigmoid(z) = (tanh(z/2)+1)/2), keep cell/hidden
    state doubled (C2=2c, H2=2h), pre-scale Whh / lin_w accordingly; gate
    order permuted to [i, f, o, g].
  - x is padded to [B, T, 128] bf16 (dma_start_transpose needs 128 output
    partitions); bias rides the recurrent matmul via a constant-ones row of
    the hidden-state buffer. A small first time-chunk lets compute start
    before the bulk of x lands.
  - Device per step, per chain (chain = 2 batch groups of 128):
      PE:  per group, one x-side matmul (lhsT = x^T slice) + four 32-row
           block matmuls (lhsT = DVE-block-transposed H2, rhs = [Whh; bias]
           replicated per 32-block) accumulate gates in PSUM.
      ACT: one tanh over all gates -> packed [Ti Tf To Tg | c2] buffer.
      DVE: STT12/STT3 update the cell; ACT tanh(c2/2); STT4 forms H2 into a
           32-padded buffer whose col 10 is constant 1.0 (bias row); a DVE
           32x32 block transpose produces next step's matmul operand in SBUF
           directly -- no PE transpose, no PSUM->SBUF copy.
  - Two chains interleave so engine work of one overlaps the serial
    dependency chain of the other.
"""
import os
import sys

sys.path.insert(0, "/opt/trn_rl_repo")

import numpy as np
import ml_dtypes
from contextlib import ExitStack

import concourse.bass as bass
import concourse.bacc as bacc
import concourse.tile as tile
from concourse import mybir
from concourse.bass_utils import run_bass_kernel_spmd

F32 = mybir.dt.float32
BF16 = mybir.dt.bfloat16
AF = mybir.ActivationFunctionType
OP = mybir.AluOpType

B, T, IN, H = 4096, 200, 79, 10
NCORES = 8
BC = B // NCORES            # 512 batch per core
NG = BC // 128              # 4 groups of 128
G4 = 4 * H                  # 40 gate columns per group
IN_PAD = 128                # dma_start_transpose needs 128 out partitions
TC0 = 10                    # small first chunk so compute starts early
TCN = 38                    # steady-state chunk length
NCH = 2                     # chains (independent batch halves)
GPC = NG // NCH             # groups per chain

_prog_cache = {}
LAST_RESULTS = None
LAST_IN_MAPS = None


def _softplus(v):
    return np.log1p(np.exp(-np.abs(v))) + np.maximum(v, 0.0)


def _build_program(repeat=1, chains=NCH, ablate=""):
    gpc = NG // chains
    nc = bacc.Bacc("TRN2", target_bir_lowering=False, debug=False,
                   enable_asserts=False, num_devices=NCORES)

    xp_ap = nc.dram_tensor("xp", [BC, T, IN_PAD], BF16,
                           kind="ExternalInput").ap()
    wih_ap = nc.dram_tensor("wih", [IN_PAD, G4], BF16,
                            kind="ExternalInput").ap()
    wb_ap = nc.dram_tensor("wbrep", [128, G4], BF16,
                           kind="ExternalInput").ap()
    lwb_ap = nc.dram_tensor("lwb", [128, 1], BF16, kind="ExternalInput").ap()
    out_ap = nc.dram_tensor("out", [BC, 1], F32, kind="ExternalOutput").ap()

    GBW = 5 * H                 # packed group width [Ti Tf To Tg c2]
    with tile.TileContext(nc) as tc, ExitStack() as ctx:
        const = ctx.enter_context(tc.tile_pool(name="const", bufs=1))
        xpool = ctx.enter_context(tc.tile_pool(name="xpool", bufs=2))
        work = ctx.enter_context(tc.tile_pool(name="work", bufs=3))
        state = ctx.enter_context(tc.tile_pool(name="state", bufs=1))
        psg = ctx.enter_context(tc.tile_pool(name="psg", bufs=2, space="PSUM"))

        wih_sb = const.tile([IN_PAD, G4], BF16)
        nc.sync.dma_start(out=wih_sb, in_=wih_ap)
        wb_sb = const.tile([128, G4], BF16)
        nc.sync.dma_start(out=wb_sb, in_=wb_ap)
        lwb_sb = const.tile([128, 1], BF16)
        nc.sync.dma_start(out=lwb_sb, in_=lwb_ap)

        gb = [state.tile([128, gpc * GBW], F32, tag=f"gb{q}", name=f"gb{q}")
              for q in range(chains)]
        h2b = [state.tile([128, gpc * 32], BF16, tag=f"h2b{q}", name=f"h2b{q}")
               for q in range(chains)]
        h2tb = [state.tile([128, gpc * 32], BF16, tag=f"h2tb{q}",
                           name=f"h2tb{q}") for q in range(chains)]
        for q in range(chains):
            nc.vector.memset(gb[q], 0.0)
            nc.vector.memset(h2b[q], 0.0)
            for j in range(gpc):
                nc.vector.memset(h2b[q][:, 32 * j + H:32 * j + H + 1], 1.0)
            nc.vector.transpose(h2tb[q], h2b[q])

        chunks = [(0, TC0)] + [(TC0 + TCN * i, TCN)
                               for i in range((T - TC0) // TCN)]
        for _rep in range(repeat):
            for (t0, tcn) in chunks:
                xt = xpool.tile([IN_PAD, tcn, BC], BF16, tag=f"xt{tcn}")
                nc.sync.dma_start_transpose(
                    out=xt,
                    in_=xp_ap[:, t0:t0 + tcn, :].rearrange(
                        "b t i -> b (t i)"))
                for tl in range(tcn):
                    for q in range(chains):
                        ps = psg.tile([128, gpc * G4], F32, tag=f"ps{q}")
                        for j in range(gpc):
                            I = q * gpc + j
                            nc.tensor.matmul(
                                ps[:, G4 * j:G4 * j + G4],
                                lhsT=xt[0:IN_PAD, tl, 128 * I:128 * I + 128],
                                rhs=wih_sb, start=True,
                                stop=("noh" in ablate),
                                skip_group_check=True)
                            if "noh" not in ablate:
                                for k in range(4):
                                    nc.tensor.matmul(
                                        ps[32 * k:32 * k + 32,
                                           G4 * j:G4 * j + G4],
                                        lhsT=h2tb[q][32 * k:32 * k + H + 1,
                                                     32 * j:32 * j + 32],
                                        rhs=wb_sb[32 * k:32 * k + H + 1, :],
                                        start=False, stop=(k == 3),
                                        skip_group_check=True,
                                        tile_position=(32 * k, 32 * k))

                        g = gb[q].rearrange("p (g c) -> p g c", g=gpc)
                        psv = ps.rearrange("p (g c) -> p g c", g=gpc)
                        # ACT1: all 4 gate tanh -> packed cols 0..40/group
                        nc.scalar.activation(g[:, :, 0:G4], psv, AF.Tanh)
                        if "justx" in ablate:
                            continue
                        # STT12: [Q|P] = ([Ti|Tf] + 1) * [Tg|c2]
                        pq = work.tile([128, gpc * 2 * H], F32, tag=f"pq{q}")
                        pqv = pq.rearrange("p (g c) -> p g c", g=gpc)
                        nc.vector.scalar_tensor_tensor(
                            pqv, g[:, :, 0:2 * H], 1.0, g[:, :, 3 * H:5 * H],
                            op0=OP.add, op1=OP.mult)
                        # STT3: c2' = 0.5*P + Q
                        nc.vector.scalar_tensor_tensor(
                            g[:, :, 4 * H:5 * H], pqv[:, :, H:2 * H], 0.5,
                            pqv[:, :, 0:H], op0=OP.mult, op1=OP.add)
                        # ACT2: th = tanh(c2'/2)
                        th = work.tile([128, gpc * H], F32, tag=f"th{q}")
                        thv = th.rearrange("p (g c) -> p g c", g=gpc)
                        nc.scalar.activation(thv, g[:, :, 4 * H:5 * H],
                                             AF.Tanh, scale=0.5)
                        # STT4: H2 = (To + 1) * th -> h2b cols 0..10 per block
                        h2bv = h2b[q].rearrange("p (g c) -> p g c", g=gpc)
                        nc.vector.scalar_tensor_tensor(
                            h2bv[:, :, 0:H], g[:, :, 2 * H:3 * H], 1.0, thv,
                            op0=OP.add, op1=OP.mult)
                        if "notr" not in ablate:
                            # DVE 32x32 block transpose -> next matmul operand
                            nc.vector.transpose(h2tb[q], h2b[q])

        # linear head: out[b] = sum_h H2[b,h] * (lw[h]/2)
        ps_o = psg.tile([128, NG], F32, tag="pso")
        for q in range(chains):
            for j in range(gpc):
                I = q * gpc + j
                for k in range(4):
                    nc.tensor.matmul(
                        ps_o[32 * k:32 * k + 32, I:I + 1],
                        lhsT=h2tb[q][32 * k:32 * k + H + 1,
                                     32 * j:32 * j + 32],
                        rhs=lwb_sb[32 * k:32 * k + H + 1, :],
                        start=True, stop=True, skip_group_check=True,
                        tile_position=(32 * k, 32 * k))
        o_sb = work.tile([128, NG], F32, tag="osb")
        nc.vector.tensor_copy(o_sb, ps_o)
        nc.sync.dma_start(
            out=out_ap.rearrange("(i p) o -> p (i o)", p=128), in_=o_sb)

    nc.compile()
    return nc


def _host_weights(wih_mu, wih_rho, wih_eps, whh_mu, whh_rho, whh_eps,
                  b_mu, b_rho, b_eps, lin_w):
    Wih = (np.asarray(wih_mu, np.float32)
           + _softplus(np.asarray(wih_rho, np.float32))
           * np.asarray(wih_eps, np.float32))
    Whh = (np.asarray(whh_mu, np.float32)
           + _softplus(np.asarray(whh_rho, np.float32))
           * np.asarray(whh_eps, np.float32))
    bb = (np.asarray(b_mu, np.float32)
          + _softplus(np.asarray(b_rho, np.float32))
          * np.asarray(b_eps, np.float32))

    # permute gates from reference order [i f g o] to [i f o g]
    perm = np.r_[0:H, H:2 * H, 3 * H:4 * H, 2 * H:3 * H]
    # tanh folding: i,f,o scaled by 0.5, g unscaled
    s = np.ones(G4, np.float32) * 0.5
    s[3 * H:4 * H] = 1.0
    Wih_p = Wih[:, perm] * s
    Whh_p = Whh[:, perm] * s * 0.5     # fed H2 = 2h
    bb_p = bb[perm] * s

    wih_pad = np.zeros((IN_PAD, G4), np.float32)
    wih_pad[:IN, :] = Wih_p

    wb_rep = np.zeros((128, G4), np.float32)
    for k in range(4):
        wb_rep[32 * k:32 * k + H, :] = Whh_p
        wb_rep[32 * k + H, :] = bb_p

    lwb = np.zeros((128, 1), np.float32)
    for k in range(4):
        lwb[32 * k:32 * k + H, 0] = np.asarray(lin_w, np.float32)[:, 0] * 0.5
    return (wih_pad.astype(ml_dtypes.bfloat16),
            wb_rep.astype(ml_dtypes.bfloat16),
            lwb.astype(ml_dtypes.bfloat16))


def kernel(x, wih_mu, wih_rho, wih_eps, whh_mu, whh_rho, whh_eps,
           b_mu, b_rho, b_eps, lin_w, lin_b):
    global LAST_RESULTS, LAST_IN_MAPS
    x = np.asarray(x, np.float32)
    wih_b, wb_b, lwb_b = _host_weights(
        wih_mu, wih_rho, wih_eps, whh_mu, whh_rho, whh_eps,
        b_mu, b_rho, b_eps, lin_w)

    x_pad = np.zeros((B, T, IN_PAD), ml_dtypes.bfloat16)
    x_pad[:, :, :IN] = x.astype(ml_dtypes.bfloat16)

    if "prog" not in _prog_cache:
        _prog_cache["prog"] = _build_program(1)
    nc = _prog_cache["prog"]

    in_maps = [
        dict(xp=np.ascontiguousarray(x_pad[c * BC:(c + 1) * BC]),
             wih=wih_b, wbrep=wb_b, lwb=lwb_b)
        for c in range(NCORES)
    ]
    LAST_IN_MAPS = in_maps
    res = run_bass_kernel_spmd(nc, in_maps, list(range(NCORES)), trace=False)
    LAST_RESULTS = res
    out = np.concatenate([res.results[c]["out"] for c in range(NCORES)], 0)
    return out + np.float32(np.asarray(lin_b, np.float32)[0])



# revision 21
# speedup vs baseline: 1.8483x; 1.8483x over previous
"""Trainium2 Bass kernel for nn_CostEstimationNet (Bayesian LSTM + linear head).

Strategy (data-parallel over batch, 8 cores, 512 batch/core):
  - Host: reparameterize weights (mu + softplus(rho)*eps); gate order permuted
    to [i, f, o, g]; sigmoid folded into tanh via 0.5 pre-scaling; hidden
    state kept doubled (H2 = 2h) with Whh pre-scaled; gate pre-activations
    additionally pre-scaled by MA (the input normalization of the polynomial
    tanh, see below); cell state kept as W = MA * c.
  - tanh is evaluated ON THE VECTOR ENGINE as a composition of two custom DVE
    ops (registered in dve_ops.OPS at import): each op computes
    u = clamp(in, +-C0); out = u * (C1 + u^2 + C2 * u^4)  -- a cubic-
    normalized clamped odd quintic. Two chained applications give a
    piecewise-polynomial tanh with max |err| 1.6e-3. A third op CELL3
    (out = 0.5 * P + (0.5 * MA) * Q) fuses the cell-state combine.
    This keeps the ENTIRE per-step elementwise chain on the DVE engine
    (zero Activation-engine hops on the recurrent critical path).
  - x is padded to [B, T, 128] bf16 and DMA-transposed in chunks so the
    input-side matmul operand x^T streams in ahead of the recurrence.
  - Device per step, per chain (chain = 2 batch groups of 128):
      PE:  per group, one x-side matmul + four 32-row block matmuls
           (lhsT = DVE-block-transposed H2, rhs = [Whh; bias] replicated
           per 32-block) accumulate gate pre-activations in PSUM.
      DVE: TA(PSUM)->TB -> gates tanh; STT12 -> (T_if+1)*[Tg|W];
           CELL3 -> new W; TA->TB -> tanh(c'); STT4 -> H2 (bf16);
           32x32 block transpose -> next step's matmul operand.
  - Two chains interleave so one chain's matmul/sem handoff overlaps the
    other chain's DVE block.
"""
import os
import sys

sys.path.insert(0, "/opt/trn_rl_repo")

import numpy as np
import ml_dtypes
from contextlib import ExitStack

import concourse.bass as bass
import concourse.bacc as bacc
import concourse.tile as tile
from concourse import mybir
from concourse.bass_utils import run_bass_kernel_spmd

F32 = mybir.dt.float32
BF16 = mybir.dt.bfloat16
AF = mybir.ActivationFunctionType
OP = mybir.AluOpType

B, T, IN, H = 4096, 200, 79, 10
NCORES = 8
BC = B // NCORES            # 512 batch per core
NG = BC // 128              # 4 groups of 128
G4 = 4 * H                  # 40 gate columns per group
IN_PAD = 128                # dma_start_transpose needs 128-col input blocks
TC0 = 10                    # small first chunk so compute starts early
TCN = 38                    # steady-state chunk length
NCH = 2                     # chains (independent batch halves)
GPC = NG // NCH             # groups per chain

# ---- polynomial tanh: tanh(x) ~= P_B(clamp(P_A(clamp(MA*x)))) ------------
# fit: tanh(x) ~= outer(clamp(inner(clamp(x, La)), Lb)) with
#   inner(x) = x*(a1 + a2 x^2 + a3 x^4), outer(w) = w*(b1 + b2 w^2 + b3 w^4)
_LA = 3.22445107
_A1, _A2, _A3 = 1.06142021, -8.52707540e-02, 3.17246976e-03
_LB = 3.11786352
_B1, _B2, _B3 = 9.34752689e-01, -1.84403969e-01, 2.28152777e-02
# normalize outer cubic to 1 (fold m_b into inner), then inner cubic to 1
# (fold m_a into the host-side weights)
_MB = np.cbrt(_B2)
_PB_C0 = abs(_MB) * _LB                 # outer clamp (on m_b*w)
_PB_C1 = _B1 / _MB
_PB_C2 = _B3 / _MB ** 5
_A1s, _A2s, _A3s = _A1 * _MB, _A2 * _MB, _A3 * _MB
MA = float(np.cbrt(_A2s))               # host-side pre-scale of gate preacts
_PA_C0 = abs(MA) * _LA                  # inner clamp (on m_a*x)
_PA_C1 = _A1s / MA
_PA_C2 = _A3s / MA ** 5

_prog_cache = {}
LAST_RESULTS = None
LAST_IN_MAPS = None


def _register_custom_ops():
    from concourse import dve_ops
    from concourse.dve_ops import DveOp
    from concourse.dve_spec import (
        Spec, Src0, Src1, C0, C1, C2, Zero, minn, maxx, sq, lower, _has_src1,
    )
    from concourse.dve_uop import DveOpSpec

    def reg(name, body, reference):
        for o in dve_ops.OPS:
            if o.name == name:
                return o
        spec = Spec(body=body, reference=reference)
        op = DveOp(name, spec, subdim=False, uops_sha={})
        for v in ("v3", "v4"):
            s = DveOpSpec(name=name, opcode=1, uops=lower(spec, ver=v),
                          rd1_en=_has_src1(spec))
            op.uops_sha[v] = s.sha(v)
        dve_ops.OPS.append(op)
        dve_ops.CUSTOM_DVE_SPECS[name] = spec
        dve_ops._SUB_OPCODE_FOR_NAME[name] = (
            dve_ops._CUSTOM_DVE_ROW_BASE + len(dve_ops.OPS) - 1)
        return op

    u = maxx(minn(Src0, C0), Zero - C0)
    s = sq(u)
    ptanh = reg(
        "LSTM_PT5C", ((C1 + s) + C2 * sq(s)) * u,
        lambda in0, in1, s0, s1, imm2: (
            lambda uu: uu * (s1 + uu * uu + imm2 * uu ** 4)
        )(np.clip(in0, -s0, s0)),
    )
    cell3 = reg(
        "LSTM_CELL3", Src0 * C0 + Src1 * C1,
        lambda in0, in1, s0, s1, imm2: in0 * s0 + in1 * s1,
    )
    return ptanh, cell3


PT5C, CELL3 = _register_custom_ops()


def _softplus(v):
    return np.log1p(np.exp(-np.abs(v))) + np.maximum(v, 0.0)


def _build_program(repeat=1, chains=NCH, order="stage", tanh="dve",
                   ablate=""):
    gpc = NG // chains
    nc = bacc.Bacc("TRN2", target_bir_lowering=False, debug=False,
                   enable_asserts=False, num_devices=NCORES)

    xp_ap = nc.dram_tensor("xp", [BC, T, IN_PAD], BF16,
                           kind="ExternalInput").ap()
    wih_ap = nc.dram_tensor("wih", [IN_PAD, G4], BF16,
                            kind="ExternalInput").ap()
    wb_ap = nc.dram_tensor("wbrep", [128, G4], BF16,
                           kind="ExternalInput").ap()
    lwb_ap = nc.dram_tensor("lwb", [128, 1], BF16, kind="ExternalInput").ap()
    out_ap = nc.dram_tensor("out", [BC, 1], F32, kind="ExternalOutput").ap()

    GBW = 5 * H                 # packed group width [Ti Tf To Tg | W]
    with tile.TileContext(nc) as tc, ExitStack() as ctx:
        const = ctx.enter_context(tc.tile_pool(name="const", bufs=1))
        xpool = ctx.enter_context(tc.tile_pool(name="xpool", bufs=2))
        work = ctx.enter_context(tc.tile_pool(name="work", bufs=3))
        state = ctx.enter_context(tc.tile_pool(name="state", bufs=1))
        psg = ctx.enter_context(tc.tile_pool(name="psg", bufs=2, space="PSUM"))

        wih_sb = const.tile([IN_PAD, G4], BF16)
        nc.sync.dma_start(out=wih_sb, in_=wih_ap)
        wb_sb = const.tile([128, G4], BF16)
        nc.sync.dma_start(out=wb_sb, in_=wb_ap)
        lwb_sb = const.tile([128, 1], BF16)
        nc.sync.dma_start(out=lwb_sb, in_=lwb_ap)

        gb = [state.tile([128, gpc * GBW], F32, tag=f"gb{q}", name=f"gb{q}")
              for q in range(chains)]
        h2b = [state.tile([128, gpc * 32], BF16, tag=f"h2b{q}", name=f"h2b{q}")
               for q in range(chains)]
        h2tb = [state.tile([128, gpc * 32], BF16, tag=f"h2tb{q}",
                           name=f"h2tb{q}") for q in range(chains)]
        ta = [state.tile([128, gpc * G4], F32, tag=f"ta{q}", name=f"ta{q}")
              for q in range(chains)]
        tb = [state.tile([128, gpc * H], F32, tag=f"tb{q}", name=f"tb{q}")
              for q in range(chains)]
        th = [state.tile([128, gpc * H], F32, tag=f"th{q}", name=f"th{q}")
              for q in range(chains)]
        for q in range(chains):
            nc.vector.memset(gb[q], 0.0)
            nc.vector.memset(h2b[q], 0.0)
            for j in range(gpc):
                nc.vector.memset(h2b[q][:, 32 * j + H:32 * j + H + 1], 1.0)
            nc.vector.transpose(h2tb[q], h2b[q])

        def emit_pe(q, xt, tl):
            ps = psg.tile([128, gpc * G4], F32, tag=f"ps{q}")
            noh = "noh" in ablate
            for j in range(gpc):
                I = q * gpc + j
                nc.tensor.matmul(
                    ps[:, G4 * j:G4 * j + G4],
                    lhsT=xt[0:IN_PAD, tl, 128 * I:128 * I + 128],
                    rhs=wih_sb, start=True, stop=noh,
                    skip_group_check=True)
                if noh:
                    continue
                for k in range(4):
                    nc.tensor.matmul(
                        ps[32 * k:32 * k + 32, G4 * j:G4 * j + G4],
                        lhsT=h2tb[q][32 * k:32 * k + H + 1,
                                     32 * j:32 * j + 32],
                        rhs=wb_sb[32 * k:32 * k + H + 1, :],
                        start=False, stop=(k == 3),
                        skip_group_check=True,
                        tile_position=(32 * k, 32 * k))
            return ps

        def emit_tanh_gates(q, ps):
            g = gb[q].rearrange("p (g c) -> p g c", g=gpc)
            psv = ps.rearrange("p (g c) -> p g c", g=gpc)
            if tanh == "act":
                nc.scalar.activation(g[:, :, 0:G4], psv, AF.Tanh)
            else:
                tav = ta[q].rearrange("p (g c) -> p g c", g=gpc)
                nc.vector._custom_dve(PT5C, out=tav, in0=psv,
                                      s0=_PA_C0, s1=_PA_C1, imm2=_PA_C2)
                nc.vector._custom_dve(PT5C, out=g[:, :, 0:G4], in0=tav,
                                      s0=_PB_C0, s1=_PB_C1, imm2=_PB_C2)

        def emit_cell(q):
            # STT12: [Q|P] = ([Ti|Tf] + 1) * [Tg|W]
            g = gb[q].rearrange("p (g c) -> p g c", g=gpc)
            pq = work.tile([128, gpc * 2 * H], F32, tag=f"pq{q}")
            pqv = pq.rearrange("p (g c) -> p g c", g=gpc)
            nc.vector.scalar_tensor_tensor(
                pqv, g[:, :, 0:2 * H], 1.0, g[:, :, 3 * H:5 * H],
                op0=OP.add, op1=OP.mult)
            if tanh == "act":
                # W' = 0.5*P + Q  (state W = 2c here)
                nc.vector.scalar_tensor_tensor(
                    g[:, :, 4 * H:5 * H], pqv[:, :, H:2 * H], 0.5,
                    pqv[:, :, 0:H], op0=OP.mult, op1=OP.add)
            else:
                # CELL3: W' = 0.5*P + (0.5*MA)*Q  (state W = MA*c)
                nc.vector._custom_dve(
                    CELL3, out=g[:, :, 4 * H:5 * H],
                    in0=pqv[:, :, H:2 * H], in1=pqv[:, :, 0:H],
                    s0=0.5, s1=0.5 * MA)

        def emit_tanh_cell(q):
            g = gb[q].rearrange("p (g c) -> p g c", g=gpc)
            thv = th[q].rearrange("p (g c) -> p g c", g=gpc)
            if tanh == "act":
                nc.scalar.activation(thv, g[:, :, 4 * H:5 * H],
                                     AF.Tanh, scale=0.5)
            else:
                tbv = tb[q].rearrange("p (g c) -> p g c", g=gpc)
                nc.vector._custom_dve(PT5C, out=tbv,
                                      in0=g[:, :, 4 * H:5 * H],
                                      s0=_PA_C0, s1=_PA_C1, imm2=_PA_C2)
                nc.vector._custom_dve(PT5C, out=thv, in0=tbv,
                                      s0=_PB_C0, s1=_PB_C1, imm2=_PB_C2)

        def emit_hform(q):
            # STT4: H2 = (To + 1) * th -> h2b cols 0..10 per 32-block
            g = gb[q].rearrange("p (g c) -> p g c", g=gpc)
            thv = th[q].rearrange("p (g c) -> p g c", g=gpc)
            h2bv = h2b[q].rearrange("p (g c) -> p g c", g=gpc)
            nc.vector.scalar_tensor_tensor(
                h2bv[:, :, 0:H], g[:, :, 2 * H:3 * H], 1.0, thv,
                op0=OP.add, op1=OP.mult)
            # DVE 32x32 block transpose -> next step's matmul operand
            nc.vector.transpose(h2tb[q], h2b[q])

        chunks = [(0, TC0)] + [(TC0 + TCN * i, TCN)
                               for i in range((T - TC0) // TCN)]
        xconst = None
        if "nox" in ablate:
            xconst = const.tile([IN_PAD, 1, BC], BF16)
            nc.vector.memset(xconst, 0.01)
        for _rep in range(repeat):
            for (t0, tcn) in chunks:
                if "nox" in ablate:
                    xt = xconst.rearrange("p a b -> p (a b)").rearrange(
                        "p (t b) -> p t b", t=1)
                    tcnl = [0] * tcn
                else:
                    xt = xpool.tile([IN_PAD, tcn, BC], BF16, tag=f"xt{tcn}")
                    nc.sync.dma_start_transpose(
                        out=xt,
                        in_=xp_ap[:, t0:t0 + tcn, :].rearrange(
                            "b t i -> b (t i)"))
                    tcnl = None
                for tl in (tcnl if tcnl is not None else range(tcn)):
                    if "justx" in ablate:
                        for q in range(chains):
                            emit_pe(q, xt, tl)
                        continue
                    if order == "chain":
                        for q in range(chains):
                            ps = emit_pe(q, xt, tl)
                            emit_tanh_gates(q, ps)
                            emit_cell(q)
                            emit_tanh_cell(q)
                            emit_hform(q)
                    else:  # stage-major
                        pss = [emit_pe(q, xt, tl) for q in range(chains)]
                        for q in range(chains):
                            emit_tanh_gates(q, pss[q])
                        for q in range(chains):
                            emit_cell(q)
                        for q in range(chains):
                            emit_tanh_cell(q)
                        for q in range(chains):
                            emit_hform(q)

        # linear head: out[b] = sum_h H2[b,h] * (lw[h]/2)
        ps_o = psg.tile([128, NG], F32, tag="ps0")
        for q in range(chains):
            for j in range(gpc):
                I = q * gpc + j
                for k in range(4):
                    nc.tensor.matmul(
                        ps_o[32 * k:32 * k + 32, I:I + 1],
                        lhsT=h2tb[q][32 * k:32 * k + H + 1,
                                     32 * j:32 * j + 32],
                        rhs=lwb_sb[32 * k:32 * k + H + 1, :],
                        start=True, stop=True, skip_group_check=True,
                        tile_position=(32 * k, 32 * k))
        o_sb = work.tile([128, NG], F32, tag="osb")
        nc.vector.tensor_copy(o_sb, ps_o)
        nc.sync.dma_start(
            out=out_ap.rearrange("(i p) o -> p (i o)", p=128), in_=o_sb)

    nc.compile()
    return nc


def _host_weights(wih_mu, wih_rho, wih_eps, whh_mu, whh_rho, whh_eps,
                  b_mu, b_rho, b_eps, lin_w, tanh="dve"):
    Wih = (np.asarray(wih_mu, np.float32)
           + _softplus(np.asarray(wih_rho, np.float32))
           * np.asarray(wih_eps, np.float32))
    Whh = (np.asarray(whh_mu, np.float32)
           + _softplus(np.asarray(whh_rho, np.float32))
           * np.asarray(whh_eps, np.float32))
    bb = (np.asarray(b_mu, np.float32)
          + _softplus(np.asarray(b_rho, np.float32))
          * np.asarray(b_eps, np.float32))

    # permute gates from reference order [i f g o] to [i f o g]
    perm = np.r_[0:H, H:2 * H, 3 * H:4 * H, 2 * H:3 * H]
    # tanh folding: i,f,o scaled by 0.5, g unscaled; all scaled by MA for
    # the polynomial tanh input normalization
    s = np.ones(G4, np.float32) * 0.5
    s[3 * H:4 * H] = 1.0
    if tanh == "dve":
        s *= MA
    Wih_p = Wih[:, perm] * s
    Whh_p = Whh[:, perm] * s * 0.5     # fed H2 = 2h
    bb_p = bb[perm] * s

    wih_pad = np.zeros((IN_PAD, G4), np.float32)
    wih_pad[:IN, :] = Wih_p

    wb_rep = np.zeros((128, G4), np.float32)
    for k in range(4):
        wb_rep[32 * k:32 * k + H, :] = Whh_p
        wb_rep[32 * k + H, :] = bb_p

    lwb = np.zeros((128, 1), np.float32)
    for k in range(4):
        lwb[32 * k:32 * k + H, 0] = np.asarray(lin_w, np.float32)[:, 0] * 0.5
    return (wih_pad.astype(ml_dtypes.bfloat16),
            wb_rep.astype(ml_dtypes.bfloat16),
            lwb.astype(ml_dtypes.bfloat16))


def _build_program_raw(repeat=1, chains=NCH, **_ignored):
    """Hand-synchronized (no TileContext) build: one inline semaphore wait
    per dependent instruction, ping-pong buffers, no standalone sem-wait
    instructions on compute engines. Sync graph per chain q and step t:
      hMM(q,t)   waits s_h[q] >= t+1   (DVET of t-1; init transpose gives 1)
      ACT1(q,t)  waits s_ps[q] >= t+1  (last hMM of step t)
      STT12(q,t) waits s_g[q] >= t+1   (ACT1)
      ACT2(q,t)  waits s_c[q] >= t+1   (STT3)
      STT4(q,t)  waits s_th[q] >= t+1  (ACT2)
    All other ordering (incl. every WAR hazard) is implied transitively by
    engine program order plus the chain above; see inline notes.
    """
    gpc = NG // chains
    nc = bacc.Bacc("TRN2", target_bir_lowering=False, debug=False,
                   enable_asserts=False, num_devices=NCORES)

    xp_ap = nc.dram_tensor("xp", [BC, T, IN_PAD], BF16,
                           kind="ExternalInput").ap()
    wih_ap = nc.dram_tensor("wih", [IN_PAD, G4], BF16,
                            kind="ExternalInput").ap()
    wb_ap = nc.dram_tensor("wbrep", [128, G4], BF16,
                           kind="ExternalInput").ap()
    lwb_ap = nc.dram_tensor("lwb", [128, 1], BF16, kind="ExternalInput").ap()
    out_ap = nc.dram_tensor("out", [BC, 1], F32, kind="ExternalOutput").ap()

    GBW = 5 * H
    with ExitStack() as ctx:
        def sb(name, shape, dtype):
            return ctx.enter_context(nc.sbuf_tensor(name, shape, dtype))

        def ps_alloc(name, shape):
            return ctx.enter_context(nc.psum_tensor(name, shape, F32))

        sem = {}
        for nm in (["w", "x", "o", "oc", "od"]
                   + [f"{k}{q}" for q in range(chains)
                      for k in ("ps", "g", "c", "th", "h")]):
            sem[nm] = nc.alloc_semaphore(f"s_{nm}")

        wih_sb = sb("wih_sb", [IN_PAD, G4], BF16)
        wb_sb = sb("wb_sb", [128, G4], BF16)
        lwb_sb = sb("lwb_sb", [128, 1], BF16)
        nc.sync.dma_start(out=wih_sb.ap(), in_=wih_ap).then_inc(sem["w"], 16)
        nc.sync.dma_start(out=wb_sb.ap(), in_=wb_ap).then_inc(sem["w"], 16)
        nc.sync.dma_start(out=lwb_sb.ap(), in_=lwb_ap).then_inc(sem["w"], 16)

        gb = [sb(f"gb{q}", [128, gpc * GBW], F32) for q in range(chains)]
        h2b = [sb(f"h2b{q}", [128, gpc * 32], BF16) for q in range(chains)]
        h2tb = [sb(f"h2tb{q}", [128, gpc * 32], BF16) for q in range(chains)]
        pq = [sb(f"pq{q}", [128, gpc * 2 * H], F32) for q in range(chains)]
        th = [sb(f"th{q}", [128, gpc * H], F32) for q in range(chains)]
        xt = [sb(f"xt{i}", [IN_PAD, TCN, BC], BF16) for i in range(2)]
        o_sb = sb("o_sb", [128, NG], F32)
        psb = [[ps_alloc(f"ps{q}_{p}", [128, gpc * G4]) for p in range(2)]
               for q in range(chains)]
        ps_o = ps_alloc("ps_o", [128, NG])

        for q in range(chains):
            nc.vector.memset(gb[q].ap(), 0.0)
            nc.vector.memset(h2b[q].ap(), 0.0)
            for j in range(gpc):
                nc.vector.memset(
                    h2b[q].ap()[:, 32 * j + H:32 * j + H + 1], 1.0)
            nc.vector.transpose(h2tb[q].ap(), h2b[q].ap()).then_inc(
                sem[f"h{q}"])

        chunks = [(0, TC0)] + [(TC0 + TCN * i, TCN)
                               for i in range((T - TC0) // TCN)]
        nch = len(chunks)

        def chunk_of(t):
            return 0 if t < TC0 else 1 + (t - TC0) // TCN

        def emit_chunk_dma(rep, ci, war_gstep=None):
            t0, tcn = chunks[ci]
            buf = xt[ci % 2]
            if war_gstep is not None:
                # WAR: wait until every step that reads the old contents of
                # this buffer has completed (standalone waits on SP)
                for q in range(chains):
                    nc.sync.wait_ge(sem[f"ps{q}"], war_gstep)
            d = nc.sync.dma_start_transpose(
                out=buf.ap()[:, 0:tcn, :],
                in_=xp_ap[:, t0:t0 + tcn, :].rearrange("b t i -> b (t i)"))
            d.then_inc(sem["x"], 16)

        def emit_xmm(q, buf, tl, g, first_waits=()):
            p = psb[q][g % 2]
            for (i, (s, v)) in enumerate(first_waits):
                if i > 0:
                    nc.tensor.wait_ge(s, v)
            ret = None
            for j in range(gpc):
                I = q * gpc + j
                r = nc.tensor.matmul(
                    p.ap()[:, G4 * j:G4 * j + G4],
                    lhsT=buf.ap()[0:IN_PAD, tl, 128 * I:128 * I + 128],
                    rhs=wih_sb.ap(), start=True, stop=False,
                    skip_group_check=True)
                if ret is None:
                    ret = r
                    if first_waits:
                        s, v = first_waits[0]
                        r.wait_op(s, v, "sem-ge")
            return ret

        def emit_hmm(q, g):
            p = psb[q][g % 2]
            first = None
            last = None
            for j in range(gpc):
                for k in range(4):
                    last = nc.tensor.matmul(
                        p.ap()[32 * k:32 * k + 32, G4 * j:G4 * j + G4],
                        lhsT=h2tb[q].ap()[32 * k:32 * k + H + 1,
                                          32 * j:32 * j + 32],
                        rhs=wb_sb.ap()[32 * k:32 * k + H + 1, :],
                        start=False, stop=(j == gpc - 1 and k == 3),
                        skip_group_check=True,
                        tile_position=(32 * k, 32 * k))
                    if first is None:
                        first = last
            first.wait_op(sem[f"h{q}"], g + 1, "sem-ge")
            last.then_inc(sem[f"ps{q}"])

        total = repeat * T
        # prologue: first two chunk DMAs and step 0's x-side matmuls
        emit_chunk_dma(0, 0)
        emit_chunk_dma(0, 1)
        for q in range(chains):
            emit_xmm(q, xt[0], 0, 0,
                     first_waits=[(sem["w"], 48), (sem["x"], 16)]
                     if q == 0 else ())

        for g in range(total):
            rep, t = divmod(g, T)
            ci = chunk_of(t)
            t0, tcn = chunks[ci]
            if t == t0 and g > 0:
                # entering chunk (rep, ci): issue the DMA for the next chunk
                # (double buffer). Its buffer was last used by the chunk
                # before the current one, fully consumed once all steps
                # before rep*T + t0 completed.
                frep, fci = (rep, ci + 1) if ci + 1 < nch else (rep + 1, 0)
                if frep < repeat:
                    emit_chunk_dma(frep, fci, war_gstep=rep * T + t0)

            # PE: recurrent matmuls for step g, then x-side for step g+1
            for q in range(chains):
                emit_hmm(q, g)
            if g + 1 < total:
                nrep, nt = divmod(g + 1, T)
                nci = chunk_of(nt)
                nt0, _ = chunks[nci]
                fw = []
                if nci != ci or nt == 0:
                    fw.append((sem["x"], 16 * (nrep * nch + nci + 1)))
                for q in range(chains):
                    emit_xmm(q, xt[nci % 2], nt - nt0, g + 1,
                             first_waits=fw if q == 0 else ())

            # ACT: tanh of gates, then tanh of cell
            for q in range(chains):
                gq = gb[q].ap().rearrange("p (g c) -> p g c", g=gpc)
                pv = psb[q][g % 2].ap().rearrange("p (g c) -> p g c", g=gpc)
                nc.scalar.activation(gq[:, :, 0:G4], pv, AF.Tanh) \
                    .wait_op(sem[f"ps{q}"], g + 1, "sem-ge") \
                    .then_inc(sem[f"g{q}"])
            for q in range(chains):
                gq = gb[q].ap().rearrange("p (g c) -> p g c", g=gpc)
                thv = th[q].ap().rearrange("p (g c) -> p g c", g=gpc)
                nc.scalar.activation(thv, gq[:, :, 4 * H:5 * H],
                                     AF.Tanh, scale=0.5) \
                    .wait_op(sem[f"c{q}"], g + 1, "sem-ge") \
                    .then_inc(sem[f"th{q}"])

            # DVE: cell update, then h2 formation + transpose
            for q in range(chains):
                gq = gb[q].ap().rearrange("p (g c) -> p g c", g=gpc)
                pqv = pq[q].ap().rearrange("p (g c) -> p g c", g=gpc)
                nc.vector.scalar_tensor_tensor(
                    pqv, gq[:, :, 0:2 * H], 1.0, gq[:, :, 3 * H:5 * H],
                    op0=OP.add, op1=OP.mult) \
                    .wait_op(sem[f"g{q}"], g + 1, "sem-ge")
                nc.vector.scalar_tensor_tensor(
                    gq[:, :, 4 * H:5 * H], pqv[:, :, H:2 * H], 0.5,
                    pqv[:, :, 0:H], op0=OP.mult, op1=OP.add) \
                    .then_inc(sem[f"c{q}"])
            for q in range(chains):
                gq = gb[q].ap().rearrange("p (g c) -> p g c", g=gpc)
                thv = th[q].ap().rearrange("p (g c) -> p g c", g=gpc)
                h2bv = h2b[q].ap().rearrange("p (g c) -> p g c", g=gpc)
                nc.vector.scalar_tensor_tensor(
                    h2bv[:, :, 0:H], gq[:, :, 2 * H:3 * H], 1.0, thv,
                    op0=OP.add, op1=OP.mult) \
                    .wait_op(sem[f"th{q}"], g + 1, "sem-ge")
                nc.vector.transpose(h2tb[q].ap(), h2b[q].ap()) \
                    .then_inc(sem[f"h{q}"])

        # linear head
        for q in range(1, chains):
            nc.tensor.wait_ge(sem[f"h{q}"], total + 1)
        first = None
        last = None
        for q in range(chains):
            for j in range(gpc):
                I = q * gpc + j
                for k in range(4):
                    last = nc.tensor.matmul(
                        ps_o.ap()[32 * k:32 * k + 32, I:I + 1],
                        lhsT=h2tb[q].ap()[32 * k:32 * k + H + 1,
                                          32 * j:32 * j + 32],
                        rhs=lwb_sb.ap()[32 * k:32 * k + H + 1, :],
                        start=True, stop=True, skip_group_check=True,
                        tile_position=(32 * k, 32 * k))
                    if first is None:
                        first = last
        first.wait_op(sem["h0"], total + 1, "sem-ge")
        last.then_inc(sem["o"])
        nc.vector.tensor_copy(o_sb.ap(), ps_o.ap()) \
            .wait_op(sem["o"], 1, "sem-ge").then_inc(sem["oc"])
        with nc.allow_non_contiguous_dma(reason="tiny [128,4] out"):
            nc.sync.dma_start(
                out=out_ap.rearrange("(i p) o -> p (i o)", p=128),
                in_=o_sb.ap()) \
                .wait_op(sem["oc"], 1, "sem-ge").then_inc(sem["od"], 16)
        nc.sync.wait_ge(sem["od"], 16)
        nc.all_engine_barrier()

    nc.compile()
    return nc


def build_program(repeat=1, chains=NCH, order="stage", tanh="dve"):
    if order == "raw":
        return _build_program_raw(repeat, chains=chains)
    return _build_program(repeat, chains=chains, order=order, tanh=tanh)


# chosen configuration (sim-swept); order="raw" uses the hand-synchronized
# builder (tanh must then be "act" for the host-side weight prep)
CONFIG = dict(chains=NCH, order="chain", tanh="act")


def kernel(x, wih_mu, wih_rho, wih_eps, whh_mu, whh_rho, whh_eps,
           b_mu, b_rho, b_eps, lin_w, lin_b):
    global LAST_RESULTS, LAST_IN_MAPS
    x = np.asarray(x, np.float32)
    wih_b, wb_b, lwb_b = _host_weights(
        wih_mu, wih_rho, wih_eps, whh_mu, whh_rho, whh_eps,
        b_mu, b_rho, b_eps, lin_w, tanh=CONFIG["tanh"])

    x_pad = np.zeros((B, T, IN_PAD), ml_dtypes.bfloat16)
    x_pad[:, :, :IN] = x.astype(ml_dtypes.bfloat16)

    if "prog" not in _prog_cache:
        _prog_cache["prog"] = build_program(1, **CONFIG)
    nc = _prog_cache["prog"]

    in_maps = [
        dict(xp=np.ascontiguousarray(x_pad[c * BC:(c + 1) * BC]),
             wih=wih_b, wbrep=wb_b, lwb=lwb_b)
        for c in range(NCORES)
    ]
    LAST_IN_MAPS = in_maps
    res = run_bass_kernel_spmd(nc, in_maps, list(range(NCORES)), trace=False)
    LAST_RESULTS = res
    out = np.concatenate([res.results[c]["out"] for c in range(NCORES)], 0)
    return out + np.float32(np.asarray(lin_b, np.float32)[0])


# revision 23
# speedup vs baseline: 3.8823x; 2.1005x over previous
"""Trainium2 Bass kernel for nn_CostEstimationNet (Bayesian LSTM + linear head).

Strategy (data-parallel over batch, 8 cores, 512 batch/core):
  - Host: reparameterize weights (mu + softplus(rho)*eps); gate order permuted
    to [i, f, o, g]; sigmoid folded into tanh via 0.5 pre-scaling; hidden
    state kept doubled (H2 = 2h) with Whh pre-scaled; gate pre-activations
    additionally pre-scaled by MA (the input normalization of the polynomial
    tanh, see below); cell state kept as W = MA * c.
  - tanh is evaluated ON THE VECTOR ENGINE as a composition of two custom DVE
    ops (registered in dve_ops.OPS at import): each op computes
    u = clamp(in, +-C0); out = u * (C1 + u^2 + C2 * u^4)  -- a cubic-
    normalized clamped odd quintic. Two chained applications give a
    piecewise-polynomial tanh with max |err| 1.6e-3. A third op CELL3
    (out = 0.5 * P + (0.5 * MA) * Q) fuses the cell-state combine.
    This keeps the ENTIRE per-step elementwise chain on the DVE engine
    (zero Activation-engine hops on the recurrent critical path).
  - x is padded to [B, T, 128] bf16 and DMA-transposed in chunks so the
    input-side matmul operand x^T streams in ahead of the recurrence.
  - Device per step, per chain (chain = 2 batch groups of 128):
      PE:  per group, one x-side matmul + four 32-row block matmuls
           (lhsT = DVE-block-transposed H2, rhs = [Whh; bias] replicated
           per 32-block) accumulate gate pre-activations in PSUM.
      DVE: TA(PSUM)->TB -> gates tanh; STT12 -> (T_if+1)*[Tg|W];
           CELL3 -> new W; TA->TB -> tanh(c'); STT4 -> H2 (bf16);
           32x32 block transpose -> next step's matmul operand.
  - Two chains interleave so one chain's matmul/sem handoff overlaps the
    other chain's DVE block.
"""
import os
import sys

sys.path.insert(0, "/opt/trn_rl_repo")

import numpy as np
import ml_dtypes
from contextlib import ExitStack

import concourse.bass as bass
import concourse.bacc as bacc
import concourse.tile as tile
from concourse import mybir
from concourse.bass_utils import run_bass_kernel_spmd

F32 = mybir.dt.float32
BF16 = mybir.dt.bfloat16
AF = mybir.ActivationFunctionType
OP = mybir.AluOpType

B, T, IN, H = 4096, 200, 79, 10
NCORES = 8
BC = B // NCORES            # 512 batch per core
NG = BC // 128              # 4 groups of 128
G4 = 4 * H                  # 40 gate columns per group
IN_PAD = 128                # dma_start_transpose needs 128-col input blocks
TC0 = 10                    # small first chunk so compute starts early
TCN = 38                    # steady-state chunk length
NCH = 2                     # chains (independent batch halves)
GPC = NG // NCH             # groups per chain

# ---- polynomial tanh: tanh(x) ~= P_B(clamp(P_A(clamp(MA*x)))) ------------
# fit: tanh(x) ~= outer(clamp(inner(clamp(x, La)), Lb)) with
#   inner(x) = x*(a1 + a2 x^2 + a3 x^4), outer(w) = w*(b1 + b2 w^2 + b3 w^4)
_LA = 3.22445107
_A1, _A2, _A3 = 1.06142021, -8.52707540e-02, 3.17246976e-03
_LB = 3.11786352
_B1, _B2, _B3 = 9.34752689e-01, -1.84403969e-01, 2.28152777e-02
# normalize outer cubic to 1 (fold m_b into inner), then inner cubic to 1
# (fold m_a into the host-side weights)
_MB = np.cbrt(_B2)
_PB_C0 = abs(_MB) * _LB                 # outer clamp (on m_b*w)
_PB_C1 = _B1 / _MB
_PB_C2 = _B3 / _MB ** 5
_A1s, _A2s, _A3s = _A1 * _MB, _A2 * _MB, _A3 * _MB
MA = float(np.cbrt(_A2s))               # host-side pre-scale of gate preacts
_PA_C0 = abs(MA) * _LA                  # inner clamp (on m_a*x)
_PA_C1 = _A1s / MA
_PA_C2 = _A3s / MA ** 5

_prog_cache = {}
LAST_RESULTS = None
LAST_IN_MAPS = None


def _register_custom_ops():
    from concourse import dve_ops
    from concourse.dve_ops import DveOp
    from concourse.dve_spec import (
        Spec, Src0, Src1, C0, C1, C2, Zero, minn, maxx, sq, lower, _has_src1,
    )
    from concourse.dve_uop import DveOpSpec

    def reg(name, body, reference):
        for o in dve_ops.OPS:
            if o.name == name:
                return o
        spec = Spec(body=body, reference=reference)
        op = DveOp(name, spec, subdim=False, uops_sha={})
        for v in ("v3", "v4"):
            s = DveOpSpec(name=name, opcode=1, uops=lower(spec, ver=v),
                          rd1_en=_has_src1(spec))
            op.uops_sha[v] = s.sha(v)
        dve_ops.OPS.append(op)
        dve_ops.CUSTOM_DVE_SPECS[name] = spec
        dve_ops._SUB_OPCODE_FOR_NAME[name] = (
            dve_ops._CUSTOM_DVE_ROW_BASE + len(dve_ops.OPS) - 1)
        return op

    u = maxx(minn(Src0, C0), Zero - C0)
    s = sq(u)
    ptanh = reg(
        "LSTM_PT5C", ((C1 + s) + C2 * sq(s)) * u,
        lambda in0, in1, s0, s1, imm2: (
            lambda uu: uu * (s1 + uu * uu + imm2 * uu ** 4)
        )(np.clip(in0, -s0, s0)),
    )
    cell3 = reg(
        "LSTM_CELL3", Src0 * C0 + Src1 * C1,
        lambda in0, in1, s0, s1, imm2: in0 * s0 + in1 * s1,
    )
    return ptanh, cell3


PT5C, CELL3 = _register_custom_ops()


def _softplus(v):
    return np.log1p(np.exp(-np.abs(v))) + np.maximum(v, 0.0)


def _build_program(repeat=1, chains=NCH, order="stage", tanh="dve",
                   ablate=""):
    gpc = NG // chains
    nc = bacc.Bacc("TRN2", target_bir_lowering=False, debug=False,
                   enable_asserts=False, num_devices=NCORES)

    xp_ap = nc.dram_tensor("xp", [BC, T, IN_PAD], BF16,
                           kind="ExternalInput").ap()
    wih_ap = nc.dram_tensor("wih", [IN_PAD, G4], BF16,
                            kind="ExternalInput").ap()
    wb_ap = nc.dram_tensor("wbrep", [128, G4], BF16,
                           kind="ExternalInput").ap()
    lwb_ap = nc.dram_tensor("lwb", [128, 1], BF16, kind="ExternalInput").ap()
    out_ap = nc.dram_tensor("out", [BC, 1], F32, kind="ExternalOutput").ap()

    GBW = 5 * H                 # packed group width [Ti Tf To Tg | W]
    with tile.TileContext(nc) as tc, ExitStack() as ctx:
        const = ctx.enter_context(tc.tile_pool(name="const", bufs=1))
        xpool = ctx.enter_context(tc.tile_pool(name="xpool", bufs=2))
        work = ctx.enter_context(tc.tile_pool(name="work", bufs=3))
        state = ctx.enter_context(tc.tile_pool(name="state", bufs=1))
        psg = ctx.enter_context(tc.tile_pool(name="psg", bufs=2, space="PSUM"))

        wih_sb = const.tile([IN_PAD, G4], BF16)
        nc.sync.dma_start(out=wih_sb, in_=wih_ap)
        wb_sb = const.tile([128, G4], BF16)
        nc.sync.dma_start(out=wb_sb, in_=wb_ap)
        lwb_sb = const.tile([128, 1], BF16)
        nc.sync.dma_start(out=lwb_sb, in_=lwb_ap)

        gb = [state.tile([128, gpc * GBW], F32, tag=f"gb{q}", name=f"gb{q}")
              for q in range(chains)]
        h2b = [state.tile([128, gpc * 32], BF16, tag=f"h2b{q}", name=f"h2b{q}")
               for q in range(chains)]
        h2tb = [state.tile([128, gpc * 32], BF16, tag=f"h2tb{q}",
                           name=f"h2tb{q}") for q in range(chains)]
        ta = [state.tile([128, gpc * G4], F32, tag=f"ta{q}", name=f"ta{q}")
              for q in range(chains)]
        tb = [state.tile([128, gpc * H], F32, tag=f"tb{q}", name=f"tb{q}")
              for q in range(chains)]
        th = [state.tile([128, gpc * H], F32, tag=f"th{q}", name=f"th{q}")
              for q in range(chains)]
        for q in range(chains):
            nc.vector.memset(gb[q], 0.0)
            nc.vector.memset(h2b[q], 0.0)
            for j in range(gpc):
                nc.vector.memset(h2b[q][:, 32 * j + H:32 * j + H + 1], 1.0)
            nc.vector.transpose(h2tb[q], h2b[q])

        def emit_pe(q, xt, tl):
            ps = psg.tile([128, gpc * G4], F32, tag=f"ps{q}")
            noh = "noh" in ablate
            for j in range(gpc):
                I = q * gpc + j
                nc.tensor.matmul(
                    ps[:, G4 * j:G4 * j + G4],
                    lhsT=xt[0:IN_PAD, tl, 128 * I:128 * I + 128],
                    rhs=wih_sb, start=True, stop=noh,
                    skip_group_check=True)
                if noh:
                    continue
                for k in range(4):
                    nc.tensor.matmul(
                        ps[32 * k:32 * k + 32, G4 * j:G4 * j + G4],
                        lhsT=h2tb[q][32 * k:32 * k + H + 1,
                                     32 * j:32 * j + 32],
                        rhs=wb_sb[32 * k:32 * k + H + 1, :],
                        start=False, stop=(k == 3),
                        skip_group_check=True,
                        tile_position=(32 * k, 32 * k))
            return ps

        def emit_tanh_gates(q, ps):
            g = gb[q].rearrange("p (g c) -> p g c", g=gpc)
            psv = ps.rearrange("p (g c) -> p g c", g=gpc)
            if tanh == "act":
                nc.scalar.activation(g[:, :, 0:G4], psv, AF.Tanh)
            else:
                tav = ta[q].rearrange("p (g c) -> p g c", g=gpc)
                nc.vector._custom_dve(PT5C, out=tav, in0=psv,
                                      s0=_PA_C0, s1=_PA_C1, imm2=_PA_C2)
                nc.vector._custom_dve(PT5C, out=g[:, :, 0:G4], in0=tav,
                                      s0=_PB_C0, s1=_PB_C1, imm2=_PB_C2)

        def emit_cell(q):
            # STT12: [Q|P] = ([Ti|Tf] + 1) * [Tg|W]
            g = gb[q].rearrange("p (g c) -> p g c", g=gpc)
            pq = work.tile([128, gpc * 2 * H], F32, tag=f"pq{q}")
            pqv = pq.rearrange("p (g c) -> p g c", g=gpc)
            nc.vector.scalar_tensor_tensor(
                pqv, g[:, :, 0:2 * H], 1.0, g[:, :, 3 * H:5 * H],
                op0=OP.add, op1=OP.mult)
            if tanh == "act":
                # W' = 0.5*P + Q  (state W = 2c here)
                nc.vector.scalar_tensor_tensor(
                    g[:, :, 4 * H:5 * H], pqv[:, :, H:2 * H], 0.5,
                    pqv[:, :, 0:H], op0=OP.mult, op1=OP.add)
            else:
                # CELL3: W' = 0.5*P + (0.5*MA)*Q  (state W = MA*c)
                nc.vector._custom_dve(
                    CELL3, out=g[:, :, 4 * H:5 * H],
                    in0=pqv[:, :, H:2 * H], in1=pqv[:, :, 0:H],
                    s0=0.5, s1=0.5 * MA)

        def emit_tanh_cell(q):
            g = gb[q].rearrange("p (g c) -> p g c", g=gpc)
            thv = th[q].rearrange("p (g c) -> p g c", g=gpc)
            if tanh == "act":
                nc.scalar.activation(thv, g[:, :, 4 * H:5 * H],
                                     AF.Tanh, scale=0.5)
            else:
                tbv = tb[q].rearrange("p (g c) -> p g c", g=gpc)
                nc.vector._custom_dve(PT5C, out=tbv,
                                      in0=g[:, :, 4 * H:5 * H],
                                      s0=_PA_C0, s1=_PA_C1, imm2=_PA_C2)
                nc.vector._custom_dve(PT5C, out=thv, in0=tbv,
                                      s0=_PB_C0, s1=_PB_C1, imm2=_PB_C2)

        def emit_hform(q):
            # STT4: H2 = (To + 1) * th -> h2b cols 0..10 per 32-block
            g = gb[q].rearrange("p (g c) -> p g c", g=gpc)
            thv = th[q].rearrange("p (g c) -> p g c", g=gpc)
            h2bv = h2b[q].rearrange("p (g c) -> p g c", g=gpc)
            nc.vector.scalar_tensor_tensor(
                h2bv[:, :, 0:H], g[:, :, 2 * H:3 * H], 1.0, thv,
                op0=OP.add, op1=OP.mult)
            # DVE 32x32 block transpose -> next step's matmul operand
            nc.vector.transpose(h2tb[q], h2b[q])

        chunks = [(0, TC0)] + [(TC0 + TCN * i, TCN)
                               for i in range((T - TC0) // TCN)]
        xconst = None
        if "nox" in ablate:
            xconst = const.tile([IN_PAD, 1, BC], BF16)
            nc.vector.memset(xconst, 0.01)
        for _rep in range(repeat):
            for (t0, tcn) in chunks:
                if "nox" in ablate:
                    xt = xconst.rearrange("p a b -> p (a b)").rearrange(
                        "p (t b) -> p t b", t=1)
                    tcnl = [0] * tcn
                else:
                    xt = xpool.tile([IN_PAD, tcn, BC], BF16, tag=f"xt{tcn}")
                    nc.sync.dma_start_transpose(
                        out=xt,
                        in_=xp_ap[:, t0:t0 + tcn, :].rearrange(
                            "b t i -> b (t i)"))
                    tcnl = None
                for tl in (tcnl if tcnl is not None else range(tcn)):
                    if "justx" in ablate:
                        for q in range(chains):
                            emit_pe(q, xt, tl)
                        continue
                    if order == "chain":
                        for q in range(chains):
                            ps = emit_pe(q, xt, tl)
                            emit_tanh_gates(q, ps)
                            emit_cell(q)
                            emit_tanh_cell(q)
                            emit_hform(q)
                    else:  # stage-major
                        pss = [emit_pe(q, xt, tl) for q in range(chains)]
                        for q in range(chains):
                            emit_tanh_gates(q, pss[q])
                        for q in range(chains):
                            emit_cell(q)
                        for q in range(chains):
                            emit_tanh_cell(q)
                        for q in range(chains):
                            emit_hform(q)

        # linear head: out[b] = sum_h H2[b,h] * (lw[h]/2)
        ps_o = psg.tile([128, NG], F32, tag="ps0")
        for q in range(chains):
            for j in range(gpc):
                I = q * gpc + j
                for k in range(4):
                    nc.tensor.matmul(
                        ps_o[32 * k:32 * k + 32, I:I + 1],
                        lhsT=h2tb[q][32 * k:32 * k + H + 1,
                                     32 * j:32 * j + 32],
                        rhs=lwb_sb[32 * k:32 * k + H + 1, :],
                        start=True, stop=True, skip_group_check=True,
                        tile_position=(32 * k, 32 * k))
        o_sb = work.tile([128, NG], F32, tag="osb")
        nc.vector.tensor_copy(o_sb, ps_o)
        nc.sync.dma_start(
            out=out_ap.rearrange("(i p) o -> p (i o)", p=128), in_=o_sb)

    nc.compile()
    return nc


def _host_weights(wih_mu, wih_rho, wih_eps, whh_mu, whh_rho, whh_eps,
                  b_mu, b_rho, b_eps, lin_w, tanh="dve"):
    Wih = (np.asarray(wih_mu, np.float32)
           + _softplus(np.asarray(wih_rho, np.float32))
           * np.asarray(wih_eps, np.float32))
    Whh = (np.asarray(whh_mu, np.float32)
           + _softplus(np.asarray(whh_rho, np.float32))
           * np.asarray(whh_eps, np.float32))
    bb = (np.asarray(b_mu, np.float32)
          + _softplus(np.asarray(b_rho, np.float32))
          * np.asarray(b_eps, np.float32))

    # permute gates from reference order [i f g o] to [i f o g]
    perm = np.r_[0:H, H:2 * H, 3 * H:4 * H, 2 * H:3 * H]
    # tanh folding: i,f,o scaled by 0.5, g unscaled; all scaled by MA for
    # the polynomial tanh input normalization
    s = np.ones(G4, np.float32) * 0.5
    s[3 * H:4 * H] = 1.0
    if tanh == "dve":
        s *= MA
    Wih_p = Wih[:, perm] * s
    Whh_p = Whh[:, perm] * s * 0.5     # fed H2 = 2h
    bb_p = bb[perm] * s

    wih_pad = np.zeros((IN_PAD, G4), np.float32)
    wih_pad[:IN, :] = Wih_p

    wb_rep = np.zeros((128, G4), np.float32)
    for k in range(4):
        wb_rep[32 * k:32 * k + H, :] = Whh_p
        wb_rep[32 * k + H, :] = bb_p

    lwb = np.zeros((128, 1), np.float32)
    for k in range(4):
        lwb[32 * k:32 * k + H, 0] = np.asarray(lin_w, np.float32)[:, 0] * 0.5
    return (wih_pad.astype(ml_dtypes.bfloat16),
            wb_rep.astype(ml_dtypes.bfloat16),
            lwb.astype(ml_dtypes.bfloat16))


def _build_program_raw(repeat=1, chains=NCH, **_ignored):
    """Hand-synchronized (no TileContext) build: one inline semaphore wait
    per dependent instruction, ping-pong buffers, no standalone sem-wait
    instructions on compute engines. Sync graph per chain q and step t:
      hMM(q,t)   waits s_h[q] >= t+1   (DVET of t-1; init transpose gives 1)
      ACT1(q,t)  waits s_ps[q] >= t+1  (last hMM of step t)
      STT12(q,t) waits s_g[q] >= t+1   (ACT1)
      ACT2(q,t)  waits s_c[q] >= t+1   (STT3)
      STT4(q,t)  waits s_th[q] >= t+1  (ACT2)
    All other ordering (incl. every WAR hazard) is implied transitively by
    engine program order plus the chain above; see inline notes.
    """
    gpc = NG // chains
    nc = bacc.Bacc("TRN2", target_bir_lowering=False, debug=False,
                   enable_asserts=False, num_devices=NCORES)

    xp_ap = nc.dram_tensor("xp", [BC, T, IN_PAD], BF16,
                           kind="ExternalInput").ap()
    wih_ap = nc.dram_tensor("wih", [IN_PAD, G4], BF16,
                            kind="ExternalInput").ap()
    wb_ap = nc.dram_tensor("wbrep", [128, G4], BF16,
                           kind="ExternalInput").ap()
    lwb_ap = nc.dram_tensor("lwb", [128, 1], BF16, kind="ExternalInput").ap()
    out_ap = nc.dram_tensor("out", [BC, 1], F32, kind="ExternalOutput").ap()

    GBW = 5 * H
    with ExitStack() as ctx:
        def sb(name, shape, dtype):
            return ctx.enter_context(nc.sbuf_tensor(name, shape, dtype))

        def ps_alloc(name, shape):
            return ctx.enter_context(nc.psum_tensor(name, shape, F32))

        sem = {}
        for nm in (["w", "x", "o", "oc", "od"]
                   + [f"{k}{q}" for q in range(chains)
                      for k in ("ps", "g", "c", "th", "h")]):
            sem[nm] = nc.alloc_semaphore(f"s_{nm}")

        wih_sb = sb("wih_sb", [IN_PAD, G4], BF16)
        wb_sb = sb("wb_sb", [128, G4], BF16)
        lwb_sb = sb("lwb_sb", [128, 1], BF16)
        nc.sync.dma_start(out=wih_sb.ap(), in_=wih_ap).then_inc(sem["w"], 16)
        nc.sync.dma_start(out=wb_sb.ap(), in_=wb_ap).then_inc(sem["w"], 16)
        nc.sync.dma_start(out=lwb_sb.ap(), in_=lwb_ap).then_inc(sem["w"], 16)

        gb = [sb(f"gb{q}", [128, gpc * GBW], F32) for q in range(chains)]
        h2b = [sb(f"h2b{q}", [128, gpc * 32], BF16) for q in range(chains)]
        h2tb = [sb(f"h2tb{q}", [128, gpc * 32], BF16) for q in range(chains)]
        pq = [sb(f"pq{q}", [128, gpc * 2 * H], F32) for q in range(chains)]
        th = [sb(f"th{q}", [128, gpc * H], F32) for q in range(chains)]
        xt = [sb(f"xt{i}", [IN_PAD, TCN, BC], BF16) for i in range(2)]
        o_sb = sb("o_sb", [128, NG], F32)
        psb = [[ps_alloc(f"ps{q}_{p}", [128, gpc * G4]) for p in range(2)]
               for q in range(chains)]
        ps_o = ps_alloc("ps_o", [128, NG])

        for q in range(chains):
            nc.vector.memset(gb[q].ap(), 0.0)
            nc.vector.memset(h2b[q].ap(), 0.0)
            for j in range(gpc):
                nc.vector.memset(
                    h2b[q].ap()[:, 32 * j + H:32 * j + H + 1], 1.0)
            nc.vector.transpose(h2tb[q].ap(), h2b[q].ap()).then_inc(
                sem[f"h{q}"])

        chunks = [(0, TC0)] + [(TC0 + TCN * i, TCN)
                               for i in range((T - TC0) // TCN)]
        nch = len(chunks)

        def chunk_of(t):
            return 0 if t < TC0 else 1 + (t - TC0) // TCN

        def emit_chunk_dma(rep, ci, war_gstep=None):
            t0, tcn = chunks[ci]
            buf = xt[ci % 2]
            if war_gstep is not None:
                # WAR: wait until every step that reads the old contents of
                # this buffer has completed (standalone waits on SP)
                for q in range(chains):
                    nc.sync.wait_ge(sem[f"ps{q}"], war_gstep)
            d = nc.sync.dma_start_transpose(
                out=buf.ap()[:, 0:tcn, :],
                in_=xp_ap[:, t0:t0 + tcn, :].rearrange("b t i -> b (t i)"))
            d.then_inc(sem["x"], 16)

        def emit_xmm(q, buf, tl, g, first_waits=()):
            p = psb[q][g % 2]
            for (i, (s, v)) in enumerate(first_waits):
                if i > 0:
                    nc.tensor.wait_ge(s, v)
            ret = None
            for j in range(gpc):
                I = q * gpc + j
                r = nc.tensor.matmul(
                    p.ap()[:, G4 * j:G4 * j + G4],
                    lhsT=buf.ap()[0:IN_PAD, tl, 128 * I:128 * I + 128],
                    rhs=wih_sb.ap(), start=True, stop=False,
                    skip_group_check=True)
                if ret is None:
                    ret = r
                    if first_waits:
                        s, v = first_waits[0]
                        r.wait_op(s, v, "sem-ge")
            return ret

        def emit_hmm(q, g):
            p = psb[q][g % 2]
            first = None
            last = None
            for j in range(gpc):
                for k in range(4):
                    last = nc.tensor.matmul(
                        p.ap()[32 * k:32 * k + 32, G4 * j:G4 * j + G4],
                        lhsT=h2tb[q].ap()[32 * k:32 * k + H + 1,
                                          32 * j:32 * j + 32],
                        rhs=wb_sb.ap()[32 * k:32 * k + H + 1, :],
                        start=False, stop=(j == gpc - 1 and k == 3),
                        skip_group_check=True,
                        tile_position=(32 * k, 32 * k))
                    if first is None:
                        first = last
            first.wait_op(sem[f"h{q}"], g + 1, "sem-ge")
            last.then_inc(sem[f"ps{q}"])

        total = repeat * T
        # prologue: first two chunk DMAs and step 0's x-side matmuls
        emit_chunk_dma(0, 0)
        emit_chunk_dma(0, 1)
        for q in range(chains):
            emit_xmm(q, xt[0], 0, 0,
                     first_waits=[(sem["w"], 48), (sem["x"], 16)]
                     if q == 0 else ())

        for g in range(total):
            rep, t = divmod(g, T)
            ci = chunk_of(t)
            t0, tcn = chunks[ci]
            if t == t0 and g > 0:
                # entering chunk (rep, ci): issue the DMA for the next chunk
                # (double buffer). Its buffer was last used by the chunk
                # before the current one, fully consumed once all steps
                # before rep*T + t0 completed.
                frep, fci = (rep, ci + 1) if ci + 1 < nch else (rep + 1, 0)
                if frep < repeat:
                    emit_chunk_dma(frep, fci, war_gstep=rep * T + t0)

            # PE: recurrent matmuls for step g, then x-side for step g+1
            for q in range(chains):
                emit_hmm(q, g)
            if g + 1 < total:
                nrep, nt = divmod(g + 1, T)
                nci = chunk_of(nt)
                nt0, _ = chunks[nci]
                fw = []
                if nci != ci or nt == 0:
                    fw.append((sem["x"], 16 * (nrep * nch + nci + 1)))
                for q in range(chains):
                    emit_xmm(q, xt[nci % 2], nt - nt0, g + 1,
                             first_waits=fw if q == 0 else ())

            # ACT: tanh of gates, then tanh of cell
            for q in range(chains):
                gq = gb[q].ap().rearrange("p (g c) -> p g c", g=gpc)
                pv = psb[q][g % 2].ap().rearrange("p (g c) -> p g c", g=gpc)
                nc.scalar.activation(gq[:, :, 0:G4], pv, AF.Tanh) \
                    .wait_op(sem[f"ps{q}"], g + 1, "sem-ge") \
                    .then_inc(sem[f"g{q}"])
            for q in range(chains):
                gq = gb[q].ap().rearrange("p (g c) -> p g c", g=gpc)
                thv = th[q].ap().rearrange("p (g c) -> p g c", g=gpc)
                nc.scalar.activation(thv, gq[:, :, 4 * H:5 * H],
                                     AF.Tanh, scale=0.5) \
                    .wait_op(sem[f"c{q}"], g + 1, "sem-ge") \
                    .then_inc(sem[f"th{q}"])

            # DVE: cell update, then h2 formation + transpose
            for q in range(chains):
                gq = gb[q].ap().rearrange("p (g c) -> p g c", g=gpc)
                pqv = pq[q].ap().rearrange("p (g c) -> p g c", g=gpc)
                nc.vector.scalar_tensor_tensor(
                    pqv, gq[:, :, 0:2 * H], 1.0, gq[:, :, 3 * H:5 * H],
                    op0=OP.add, op1=OP.mult) \
                    .wait_op(sem[f"g{q}"], g + 1, "sem-ge")
                nc.vector.scalar_tensor_tensor(
                    gq[:, :, 4 * H:5 * H], pqv[:, :, H:2 * H], 0.5,
                    pqv[:, :, 0:H], op0=OP.mult, op1=OP.add) \
                    .then_inc(sem[f"c{q}"])
            for q in range(chains):
                gq = gb[q].ap().rearrange("p (g c) -> p g c", g=gpc)
                thv = th[q].ap().rearrange("p (g c) -> p g c", g=gpc)
                h2bv = h2b[q].ap().rearrange("p (g c) -> p g c", g=gpc)
                nc.vector.scalar_tensor_tensor(
                    h2bv[:, :, 0:H], gq[:, :, 2 * H:3 * H], 1.0, thv,
                    op0=OP.add, op1=OP.mult) \
                    .wait_op(sem[f"th{q}"], g + 1, "sem-ge")
                nc.vector.transpose(h2tb[q].ap(), h2b[q].ap()) \
                    .then_inc(sem[f"h{q}"])

        # linear head
        for q in range(1, chains):
            nc.tensor.wait_ge(sem[f"h{q}"], total + 1)
        first = None
        last = None
        for q in range(chains):
            for j in range(gpc):
                I = q * gpc + j
                for k in range(4):
                    last = nc.tensor.matmul(
                        ps_o.ap()[32 * k:32 * k + 32, I:I + 1],
                        lhsT=h2tb[q].ap()[32 * k:32 * k + H + 1,
                                          32 * j:32 * j + 32],
                        rhs=lwb_sb.ap()[32 * k:32 * k + H + 1, :],
                        start=True, stop=True, skip_group_check=True,
                        tile_position=(32 * k, 32 * k))
                    if first is None:
                        first = last
        first.wait_op(sem["h0"], total + 1, "sem-ge")
        last.then_inc(sem["o"])
        nc.vector.tensor_copy(o_sb.ap(), ps_o.ap()) \
            .wait_op(sem["o"], 1, "sem-ge").then_inc(sem["oc"])
        with nc.allow_non_contiguous_dma(reason="tiny [128,4] out"):
            nc.sync.dma_start(
                out=out_ap.rearrange("(i p) o -> p (i o)", p=128),
                in_=o_sb.ap()) \
                .wait_op(sem["oc"], 1, "sem-ge").then_inc(sem["od"], 16)
        nc.sync.wait_ge(sem["od"], 16)
        nc.all_engine_barrier()

    nc.compile()
    return nc


def build_program(repeat=1, chains=NCH, order="stage", tanh="dve"):
    if order == "raw":
        return _build_program_raw(repeat, chains=chains)
    return _build_program(repeat, chains=chains, order=order, tanh=tanh)


# chosen configuration (sim-swept); order="raw" uses the hand-synchronized
# builder (tanh must then be "act" for the host-side weight prep)
CONFIG = dict(chains=NCH, order="chain", tanh="act")


def kernel(x, wih_mu, wih_rho, wih_eps, whh_mu, whh_rho, whh_eps,
           b_mu, b_rho, b_eps, lin_w, lin_b):
    global LAST_RESULTS, LAST_IN_MAPS
    x = np.asarray(x, np.float32)
    wih_b, wb_b, lwb_b = _host_weights(
        wih_mu, wih_rho, wih_eps, whh_mu, whh_rho, whh_eps,
        b_mu, b_rho, b_eps, lin_w, tanh=CONFIG["tanh"])

    x_pad = np.zeros((B, T, IN_PAD), ml_dtypes.bfloat16)
    x_pad[:, :, :IN] = x.astype(ml_dtypes.bfloat16)

    if "prog" not in _prog_cache:
        _prog_cache["prog"] = build_program(1, **CONFIG)
    nc = _prog_cache["prog"]

    in_maps = [
        dict(xp=np.ascontiguousarray(x_pad[c * BC:(c + 1) * BC]),
             wih=wih_b, wbrep=wb_b, lwb=lwb_b)
        for c in range(NCORES)
    ]
    LAST_IN_MAPS = in_maps
    res = run_bass_kernel_spmd(nc, in_maps, list(range(NCORES)), trace=False)
    LAST_RESULTS = res
    out = np.concatenate([res.results[c]["out"] for c in range(NCORES)], 0)
    return out + np.float32(np.asarray(lin_b, np.float32)[0])
